# revision 1
# baseline (speedup 1.0000x reference)
"""GCN layer kernel for Trainium2: out[b] = D^-1/2 (A[b]+I) D^-1/2 H[b] B.

Strategy (data-parallel, one graph per NeuronCore, no collectives):
  reference:  A1 = A + I;  deg = rowsum(A1);  d = rsqrt(deg)
              out = (d ⊙rows (A1 ⊙scaled) ...) == d ⊙rows (A1 @ (d ⊙rows (H @ B)))
  We use the algebraic refactoring
              P = H @ B                      [N, O]
              X = d ⊙rows P                  [N, O]
              Y = A @ X + X  (= A1 @ X)      [N, O]
              out = d ⊙rows Y                [N, O]
  which never materializes the normalized adjacency.

  On-device layout is fully transposed: the host passes AT = A[b].T and
  HT = H[b].T (pure layout prep), the device computes YT = X^T @ A^T (+ X^T
  via an identity-matmul) entirely with the PE contracting over SBUF
  partitions, and returns OT = out[b].T which the host transposes back.

  deg (= rowsum of A = colsum of AT) is computed by ones-weight matmuls that
  overlap the 16MB AT DMA stream; all matmuls use float32r (bit-exact fp32
  semantics at full PE rate, verified on hardware).
"""
import sys

sys.path.insert(0, "/opt/trn_rl_repo")

import numpy as np

B_, N_, F_, O_ = 8, 2048, 128, 128
NT = N_ // 128  # 16 slabs of 128 rows of AT
NCHUNK = 8  # DMA granularity: 2 slabs (2MB) per chunk
SPC = NT // NCHUNK  # slabs per chunk
N_CORES = 8

_CACHE = {}
LAST_RESULTS = None


def _build_program():
    import concourse.bacc as bacc
    import concourse.tile as tile
    import concourse.mybir as mybir

    f32 = mybir.dt.float32
    f32r = mybir.dt.float32r

    nc = bacc.Bacc(None, target_bir_lowering=False)
    AT = nc.dram_tensor("at", [N_, N_], f32r, kind="ExternalInput")
    HT = nc.dram_tensor("ht", [F_, N_], f32r, kind="ExternalInput")
    BW = nc.dram_tensor("bw", [F_, O_], f32r, kind="ExternalInput")
    EYE = nc.dram_tensor("eye", [128, 128], f32r, kind="ExternalInput")
    ONES = nc.dram_tensor("ones", [128, 128], f32r, kind="ExternalInput")
    ONESF = nc.dram_tensor("onesf", [128, 1], f32, kind="ExternalInput")
    OT = nc.dram_tensor("ot", [O_, N_], f32, kind="ExternalOutput")

    at_view = AT.rearrange("(s p) i -> p s i", p=128)  # [128, NT, N_]

    with tile.TileContext(nc) as tc:
        with (
            tc.tile_pool(name="const", bufs=1) as cst,
            tc.tile_pool(name="achunks", bufs=1) as ach,
            tc.tile_pool(name="small", bufs=1) as sml,
            tc.tile_pool(name="psbig", bufs=1, space="PSUM") as psb,
            tc.tile_pool(name="pssmall", bufs=2, space="PSUM") as pss,
        ):
            ht_sb = cst.tile([128, N_], f32r, tag="ht")
            bw_sb = cst.tile([128, O_], f32r, tag="bw")
            eye_sb = cst.tile([128, 128], f32r, tag="eye")
            ones_sb = cst.tile([128, 128], f32r, tag="ones")
            onesf_sb = cst.tile([128, 1], f32, tag="onesf")
            nc.sync.dma_start(out=ht_sb, in_=HT[:, :])
            nc.sync.dma_start(out=bw_sb, in_=BW[:, :])
            nc.sync.dma_start(out=eye_sb, in_=EYE[:, :])
            nc.sync.dma_start(out=ones_sb, in_=ONES[:, :])
            nc.sync.dma_start(out=onesf_sb, in_=ONESF[:, :])

            # A^T resident chunks; DMA issued up-front so SDMA streams
            # back-to-back while PE works on whatever has landed.
            at_sb = []
            for c in range(NCHUNK):
                t = ach.tile([128, SPC, N_], f32r, tag=f"at{c}")
                nc.sync.dma_start(
                    out=t, in_=at_view[:, c * SPC : (c + 1) * SPC, :]
                )
                at_sb.append(t)

            # P = H @ B, one [128, O] tile per n-slab, evacuated to SBUF.
            p_sb = sml.tile([128, NT, O_], f32, tag="p")
            for t in range(NT):
                p_ps = pss.tile([128, O_], f32, tag="pps")
                nc.tensor.matmul(
                    p_ps,
                    ht_sb[:, t * 128 : (t + 1) * 128],
                    bw_sb,
                    start=True,
                    stop=True,
                )
                nc.vector.tensor_copy(p_sb[:, t, :], p_ps)

            # deg broadcast across partitions: ones.T @ AT accumulated over
            # slabs; overlaps the AT DMA stream chunk by chunk.
            deg_ps = psb.tile([128, N_], mybir.dt.float32, tag="big")
            for c in range(NCHUNK):
                for sl in range(SPC):
                    s = c * SPC + sl
                    for ib in range(4):
                        nc.tensor.matmul(
                            deg_ps[:, ib * 512 : (ib + 1) * 512],
                            ones_sb,
                            at_sb[c][:, sl, ib * 512 : (ib + 1) * 512],
                            start=(s == 0),
                            stop=(s == NT - 1),
                        )

            # d = 1/sqrt(deg + 1), broadcast layout [128, N_]
            dbc_sb = sml.tile([128, N_], mybir.dt.float32, tag="dbc")
            nc.scalar.activation(
                out=dbc_sb,
                in_=deg_ps,
                func=mybir.ActivationFunctionType.Sqrt,
                bias=1.0,
                scale=1.0,
            )
            nc.vector.reciprocal(out=dbc_sb, in_=dbc_sb)

            # d per-partition tiles: transpose 128-chunks of row 0 of dbc
            dt_sb = sml.tile([128, NT], mybir.dt.float32, tag="dt")
            for t in range(NT):
                tp_ps = pss.tile([128, 1], mybir.dt.float32, tag="tp")
                nc.tensor.transpose(
                    tp_ps,
                    dbc_sb[0:1, t * 128 : (t + 1) * 128],
                    onesf_sb[0:1, 0:1],
                )
                nc.vector.tensor_copy(dt_sb[:, t : t + 1], tp_ps)

            # X = d ⊙rows P, rounded to f32r for the PE
            xs = []
            for t in range(NT):
                x = sml.tile([128, O_], f32r, tag=f"x{t}")
                nc.vector.tensor_scalar_mul(x, p_sb[:, t, :], dt_sb[:, t : t + 1])
                xs.append(x)

            # YT = X^T @ A^T (+ X^T via identity matmul)
            yt_ps = psb.tile([128, N_], mybir.dt.float32, tag="big")
            for t in range(NT):
                for ib in range(4):
                    nc.tensor.matmul(
                        yt_ps[:, ib * 512 : (ib + 1) * 512],
                        xs[t],
                        at_sb[t // SPC][:, t % SPC, ib * 512 : (ib + 1) * 512],
                        start=(t == 0),
                        stop=False,
                    )
            for t in range(NT):
                nc.tensor.matmul(
                    yt_ps[:, t * 128 : (t + 1) * 128],
                    xs[t],
                    eye_sb,
                    start=False,
                    stop=(t % 4 == 3),
                )

            # OT = d ⊙cols YT, then store
            for ib in range(4):
                o_sb = sml.tile([128, 512], mybir.dt.float32, tag=f"o{ib}")
                nc.vector.tensor_mul(
                    o_sb,
                    yt_ps[:, ib * 512 : (ib + 1) * 512],
                    dbc_sb[:, ib * 512 : (ib + 1) * 512],
                )
                nc.sync.dma_start(
                    out=OT[:, ib * 512 : (ib + 1) * 512], in_=o_sb
                )

    nc.compile()
    return nc


def _get_program():
    if "nc" not in _CACHE:
        _CACHE["nc"] = _build_program()
    return _CACHE["nc"]


def kernel(H, A, B):
    global LAST_RESULTS
    from concourse.bass_utils import run_bass_kernel_spmd

    nc = _get_program()

    eye = np.eye(128, dtype=np.float32)
    ones = np.ones((128, 128), dtype=np.float32)
    onesf = np.ones((128, 1), dtype=np.float32)

    in_maps = []
    for b in range(B_):
        in_maps.append(
            {
                "at": np.ascontiguousarray(np.asarray(A[b], dtype=np.float32).T),
                "ht": np.ascontiguousarray(np.asarray(H[b], dtype=np.float32).T),
                "bw": np.ascontiguousarray(np.asarray(B, dtype=np.float32)),
                "eye": eye,
                "ones": ones,
                "onesf": onesf,
            }
        )

    res = run_bass_kernel_spmd(nc, in_maps, list(range(N_CORES)))
    LAST_RESULTS = res

    out = np.empty((B_, N_, O_), dtype=np.float32)
    for b in range(B_):
        out[b] = res.results[b]["ot"].T
    return out


# revision 2
# speedup vs baseline: 1.0658x; 1.0658x over previous
"""GCN layer kernel for Trainium2: out[b] = D^-1/2 (A[b]+I) D^-1/2 H[b] B.

Data-parallel, one graph per NeuronCore, no collectives.

Refactoring (never materializes the normalized adjacency):
    P = H @ B;  X = d ⊙rows P;  Y = A @ X + X;  out = d ⊙rows Y
with d = 1/sqrt(1 + rowsum(A)).

Device works in transposed space: host passes AT = A[b].T, HT = H[b].T
(pure layout prep). PE computes YT = X^T @ A^T (+ X^T via identity matmul)
contracting over SBUF partitions, then transposes YT back to natural [n, o]
tiles on the PE so the final d-scaling is a cheap per-partition tensor_scalar
and the output DMAs out in natural layout.

deg (= rowsum A = colsum AT) comes from ones-weight matmuls overlapping the
AT DMA stream. All matmuls use float32r (full fp32 element precision at
1 cycle/row, verified on HW). rsqrt runs on a [128,16] transposed layout
(sqrt on ACT, reciprocal on DVE) — running it on the [128,2048] broadcast
costs ~13us of DVE time.
"""
import sys

sys.path.insert(0, "/opt/trn_rl_repo")

import numpy as np

B_, N_, F_, O_ = 8, 2048, 128, 128
NT = N_ // 128  # 16 slabs of AT
NCHUNK = 4  # 4 slabs (4MB) per DMA chunk
SPC = NT // NCHUNK
N_CORES = 8

_CACHE = {}
LAST_RESULTS = None


def _build_program():
    import concourse.bacc as bacc
    import concourse.tile as tile
    import concourse.mybir as mybir

    f32 = mybir.dt.float32
    f32r = mybir.dt.float32r
    AF = mybir.ActivationFunctionType

    nc = bacc.Bacc(None, target_bir_lowering=False)
    AT = nc.dram_tensor("at", [N_, N_], f32r, kind="ExternalInput")
    HT = nc.dram_tensor("ht", [F_, N_], f32r, kind="ExternalInput")
    # consts: [bw | eye | ones | ones_col]
    CST = nc.dram_tensor("consts", [128, 385], f32r, kind="ExternalInput")
    OUT = nc.dram_tensor("out", [N_, O_], f32, kind="ExternalOutput")

    at_view = AT.rearrange("(s p) i -> p s i", p=128)  # [128, NT, N_]
    out_view = OUT.rearrange("(c p) o -> p c o", p=128)  # [128, NT, O_]

    with tile.TileContext(nc) as tc:
        with (
            tc.tile_pool(name="const", bufs=1) as cst,
            tc.tile_pool(name="achunks", bufs=1) as ach,
            tc.tile_pool(name="small", bufs=1) as sml,
            tc.tile_pool(name="outp", bufs=2) as outp,
            tc.tile_pool(name="psbig", bufs=1, space="PSUM") as psb,
            tc.tile_pool(name="pssmall", bufs=3, space="PSUM") as pss,
        ):
            cst_sb = cst.tile([128, 385], f32r, tag="cst")
            ht_sb = cst.tile([128, N_], f32r, tag="ht")
            nc.sync.dma_start(out=cst_sb, in_=CST[:, :])
            nc.sync.dma_start(out=ht_sb, in_=HT[:, :])
            bw = cst_sb[:, 0:128]
            eye = cst_sb[:, 128:256]
            ones = cst_sb[:, 256:384]
            onesf = cst_sb[:, 384:385].bitcast(f32)
            eyef = cst_sb[:, 128:256].bitcast(f32)

            # A^T resident chunks; all DMAs issued up-front (FIFO on SP ring)
            at_sb = []
            for c in range(NCHUNK):
                t = ach.tile([128, SPC, N_], f32r, tag=f"at{c}")
                nc.sync.dma_start(out=t, in_=at_view[:, c * SPC : (c + 1) * SPC, :])
                at_sb.append(t)

            # P = H @ B, evacuated to SBUF unscaled (fp32)
            p_sb = sml.tile([128, NT, O_], f32, tag="p")
            for t in range(NT):
                p_ps = pss.tile([128, O_], f32, tag="sm")
                nc.tensor.matmul(
                    p_ps, ht_sb[:, t * 128 : (t + 1) * 128], bw, start=True, stop=True
                )
                nc.vector.tensor_copy(p_sb[:, t, :], p_ps)

            # deg broadcast: ones.T @ AT accumulated over slabs, overlaps DMA
            deg_ps = psb.tile([128, N_], f32, tag="big")
            for c in range(NCHUNK):
                for sl in range(SPC):
                    s = c * SPC + sl
                    for ib in range(4):
                        nc.tensor.matmul(
                            deg_ps[:, ib * 512 : (ib + 1) * 512],
                            ones,
                            at_sb[c][:, sl, ib * 512 : (ib + 1) * 512],
                            start=(s == 0),
                            stop=(s == NT - 1),
                        )

            # sqrt(deg+1) on row 0 -> [1, N_], then transpose 128-chunks to
            # [128, 16] and reciprocal there (multi-lane, ~0.1us)
            dgsq_sb = sml.tile([1, N_], f32, tag="dgsq")
            nc.scalar.activation(
                out=dgsq_sb, in_=deg_ps[0:1, :], func=AF.Sqrt, bias=1.0, scale=1.0
            )
            dsq_sb = sml.tile([128, NT], f32, tag="dsq")
            for t in range(NT):
                tp_ps = pss.tile([128, 1], f32, tag="sm")
                nc.tensor.transpose(
                    tp_ps, dgsq_sb[0:1, t * 128 : (t + 1) * 128], onesf[0:1, 0:1]
                )
                nc.vector.tensor_copy(dsq_sb[:, t : t + 1], tp_ps)
            d_sb = sml.tile([128, NT], f32, tag="d")
            nc.vector.reciprocal(out=d_sb, in_=dsq_sb)

            # X = d ⊙rows P  (f32r tiles, per-partition scalar mul)
            xs = []
            for t in range(NT):
                x = sml.tile([128, O_], f32r, tag=f"x{t}")
                nc.vector.tensor_scalar_mul(x, p_sb[:, t, :], d_sb[:, t : t + 1])
                xs.append(x)

            # YT = X^T @ A^T (+ X^T), i-block-outer so each 512-block's
            # epilogue overlaps the next block's matmuls
            yt_ps = psb.tile([128, N_], f32, tag="big")
            for ib in range(4):
                blk = slice(ib * 512, (ib + 1) * 512)
                for t in range(NT):
                    nc.tensor.matmul(
                        yt_ps[:, blk],
                        xs[t],
                        at_sb[t // SPC][:, t % SPC, ib * 512 : (ib + 1) * 512],
                        start=(t == 0),
                        stop=False,
                    )
                for c in range(4):
                    cc = ib * 4 + c
                    nc.tensor.matmul(
                        yt_ps[:, cc * 128 : (cc + 1) * 128],
                        xs[cc],
                        eye,
                        start=False,
                        stop=(c == 3),
                    )
                # epilogue: transpose back to natural tiles, scale by d, store
                ost = outp.tile([128, 4, O_], f32, tag="ost")
                for c in range(4):
                    cc = ib * 4 + c
                    ytc = outp.tile([128, 128], f32, tag="ytc")
                    nc.vector.tensor_copy(ytc, yt_ps[:, cc * 128 : (cc + 1) * 128])
                    yn_ps = pss.tile([128, 128], f32, tag="sm")
                    nc.tensor.transpose(yn_ps, ytc, eyef)
                    nc.vector.tensor_scalar_mul(
                        ost[:, c, :], yn_ps, d_sb[:, cc : cc + 1]
                    )
                nc.sync.dma_start(
                    out=out_view[:, ib * 4 : (ib + 1) * 4, :], in_=ost
                )

    nc.compile()
    return nc


def _get_program():
    if "nc" not in _CACHE:
        _CACHE["nc"] = _build_program()
    return _CACHE["nc"]


def _make_consts():
    c = np.zeros((128, 385), dtype=np.float32)
    c[:, 128:256] = np.eye(128, dtype=np.float32)
    c[:, 256:384] = 1.0
    c[:, 384] = 1.0
    return c


def kernel(H, A, B):
    global LAST_RESULTS
    from concourse.bass_utils import run_bass_kernel_spmd

    nc = _get_program()
    consts = _make_consts()

    in_maps = []
    for b in range(B_):
        cst = consts.copy()
        cst[:, 0:128] = np.asarray(B, dtype=np.float32)
        in_maps.append(
            {
                "at": np.ascontiguousarray(np.asarray(A[b], dtype=np.float32).T),
                "ht": np.ascontiguousarray(np.asarray(H[b], dtype=np.float32).T),
                "consts": cst,
            }
        )

    res = run_bass_kernel_spmd(nc, in_maps, list(range(N_CORES)))
    LAST_RESULTS = res

    out = np.empty((B_, N_, O_), dtype=np.float32)
    for b in range(B_):
        out[b] = res.results[b]["out"]
    return out


# revision 4
# speedup vs baseline: 1.1082x; 1.0398x over previous
"""GCN layer kernel for Trainium2: out[b] = D^-1/2 (A[b]+I) D^-1/2 H[b] B.

Data-parallel, one graph per NeuronCore, no collectives.

Refactoring (never materializes the normalized adjacency):
    P = H @ B;  X = d ⊙rows P;  Y = A @ X + X;  out = d ⊙rows Y
with d = 1/sqrt(1 + rowsum(A)).

Device works in transposed space: host passes AT = A[b].T, HT = H[b].T
(pure layout prep). PE computes YT = X^T @ A^T (+ X^T via identity matmul)
contracting over SBUF partitions, then transposes YT back to natural [n, o]
tiles on the PE so the final d-scaling is a cheap per-partition tensor_scalar
and the output DMAs out in natural layout.

deg (= rowsum A = colsum AT) comes from ones-weight matmuls overlapping the
AT DMA stream. All matmuls use float32r (full fp32 element precision at
1 cycle/row, verified on HW). rsqrt runs on a [128,16] transposed layout
(sqrt on ACT, reciprocal on DVE) — running it on the [128,2048] broadcast
costs ~13us of DVE time.
"""
import sys

sys.path.insert(0, "/opt/trn_rl_repo")

import numpy as np

B_, N_, F_, O_ = 8, 2048, 128, 128
NT = N_ // 128  # 16 slabs of AT
NCHUNK = 4  # 4 slabs (4MB) per DMA chunk
SPC = NT // NCHUNK
N_CORES = 8

_CACHE = {}
LAST_RESULTS = None


def _build_program():
    import concourse.bacc as bacc
    import concourse.tile as tile
    import concourse.mybir as mybir

    f32 = mybir.dt.float32
    f32r = mybir.dt.float32r
    AF = mybir.ActivationFunctionType

    nc = bacc.Bacc(None, target_bir_lowering=False)
    AT = nc.dram_tensor("at", [N_, N_], f32r, kind="ExternalInput")
    HT = nc.dram_tensor("ht", [F_, N_], f32r, kind="ExternalInput")
    # consts: [bw | eye | ones | ones_col]
    CST = nc.dram_tensor("consts", [128, 385], f32r, kind="ExternalInput")
    OUT = nc.dram_tensor("out", [N_, O_], f32, kind="ExternalOutput")

    at_view = AT.rearrange("(s p) i -> p s i", p=128)  # [128, NT, N_]
    out_view = OUT.rearrange("(c p) o -> p c o", p=128)  # [128, NT, O_]

    with tile.TileContext(nc) as tc:
        with (
            tc.tile_pool(name="const", bufs=1) as cst,
            tc.tile_pool(name="achunks", bufs=1) as ach,
            tc.tile_pool(name="small", bufs=1) as sml,
            tc.tile_pool(name="outp", bufs=2) as outp,
            tc.tile_pool(name="psbig", bufs=1, space="PSUM") as psb,
            tc.tile_pool(name="pssmall", bufs=3, space="PSUM") as pss,
        ):
            cst_sb = cst.tile([128, 385], f32r, tag="cst")
            ht_sb = cst.tile([128, N_], f32r, tag="ht")
            nc.sync.dma_start(out=cst_sb, in_=CST[:, :])
            nc.sync.dma_start(out=ht_sb, in_=HT[:, :])
            bw = cst_sb[:, 0:128]
            eye = cst_sb[:, 128:256]
            ones = cst_sb[:, 256:384]
            onesf = cst_sb[:, 384:385].bitcast(f32)
            eyef = cst_sb[:, 128:256].bitcast(f32)

            # A^T resident chunks; all DMAs issued up-front (FIFO on SP ring)
            at_sb = []
            for c in range(NCHUNK):
                t = ach.tile([128, SPC, N_], f32r, tag=f"at{c}")
                nc.sync.dma_start(out=t, in_=at_view[:, c * SPC : (c + 1) * SPC, :])
                at_sb.append(t)

            # P = H @ B, evacuated to SBUF unscaled (fp32)
            p_sb = sml.tile([128, NT, O_], f32, tag="p")
            for t in range(NT):
                p_ps = pss.tile([128, O_], f32, tag="sm")
                nc.tensor.matmul(
                    p_ps, ht_sb[:, t * 128 : (t + 1) * 128], bw, start=True, stop=True
                )
                nc.vector.tensor_copy(p_sb[:, t, :], p_ps)

            # deg broadcast: ones.T @ AT accumulated over slabs, overlaps DMA
            deg_ps = psb.tile([128, N_], f32, tag="big")
            for c in range(NCHUNK):
                for sl in range(SPC):
                    s = c * SPC + sl
                    for ib in range(4):
                        nc.tensor.matmul(
                            deg_ps[:, ib * 512 : (ib + 1) * 512],
                            ones,
                            at_sb[c][:, sl, ib * 512 : (ib + 1) * 512],
                            start=(s == 0),
                            stop=(s == NT - 1),
                        )

            # sqrt(deg+1) on row 0 -> [1, N_] in 512-chunks, transposes of each
            # 128-chunk pipelined behind, then one [128,16] reciprocal
            dgsq_sb = sml.tile([1, N_], f32, tag="dgsq")
            dsq_sb = sml.tile([128, NT], f32, tag="dsq")
            for q in range(4):
                nc.scalar.activation(
                    out=dgsq_sb[:, q * 512 : (q + 1) * 512],
                    in_=deg_ps[0:1, q * 512 : (q + 1) * 512],
                    func=AF.Sqrt,
                    bias=1.0,
                    scale=1.0,
                )
                for t in range(q * 4, q * 4 + 4):
                    tp_ps = pss.tile([128, 1], f32, tag="sm")
                    nc.tensor.transpose(
                        tp_ps, dgsq_sb[0:1, t * 128 : (t + 1) * 128], onesf[0:1, 0:1]
                    )
                    nc.vector.tensor_copy(dsq_sb[:, t : t + 1], tp_ps)
            d_sb = sml.tile([128, NT], f32, tag="d")
            nc.vector.reciprocal(out=d_sb, in_=dsq_sb)

            # X = d ⊙rows P  (f32r tiles, per-partition scalar mul)
            xs = []
            for t in range(NT):
                x = sml.tile([128, O_], f32r, tag=f"x{t}")
                nc.vector.tensor_scalar_mul(x, p_sb[:, t, :], d_sb[:, t : t + 1])
                xs.append(x)

            # YT = X^T @ A^T (+ X^T), i-block-outer; each block's epilogue is
            # software-pipelined one block behind so the PSUM->SBUF copies run
            # during the next block's matmuls and the PE transposes never stall
            yt_ps = psb.tile([128, N_], f32, tag="big")

            def emit_mms(ib):
                blk = slice(ib * 512, (ib + 1) * 512)
                for t in range(NT):
                    nc.tensor.matmul(
                        yt_ps[:, blk],
                        xs[t],
                        at_sb[t // SPC][:, t % SPC, ib * 512 : (ib + 1) * 512],
                        start=(t == 0),
                        stop=False,
                    )
                for c in range(4):
                    cc = ib * 4 + c
                    nc.tensor.matmul(
                        yt_ps[:, cc * 128 : (cc + 1) * 128],
                        xs[cc],
                        eye,
                        start=False,
                        stop=(c == 3),
                    )

            def emit_copies(ib):
                ytcs = []
                for c in range(4):
                    cc = ib * 4 + c
                    ytc = outp.tile([128, 128], f32, tag=f"ytc{c}")
                    nc.vector.tensor_copy(ytc, yt_ps[:, cc * 128 : (cc + 1) * 128])
                    ytcs.append(ytc)
                return ytcs

            def emit_tail(ib, ytcs):
                ost = outp.tile([128, 4, O_], f32, tag="ost")
                for c in range(4):
                    cc = ib * 4 + c
                    yn_ps = pss.tile([128, 128], f32, tag="sm")
                    nc.tensor.transpose(yn_ps, ytcs[c], eyef)
                    nc.vector.tensor_scalar_mul(
                        ost[:, c, :], yn_ps, d_sb[:, cc : cc + 1]
                    )
                nc.sync.dma_start(
                    out=out_view[:, ib * 4 : (ib + 1) * 4, :], in_=ost
                )

            pending = None  # (ib, ytcs)
            for ib in range(4):
                emit_mms(ib)
                if pending is not None:
                    emit_tail(*pending)
                pending = (ib, emit_copies(ib))
            emit_tail(*pending)

    nc.compile()
    return nc


def _get_program():
    if "nc" not in _CACHE:
        _CACHE["nc"] = _build_program()
    return _CACHE["nc"]


def _make_consts():
    c = np.zeros((128, 385), dtype=np.float32)
    c[:, 128:256] = np.eye(128, dtype=np.float32)
    c[:, 256:384] = 1.0
    c[:, 384] = 1.0
    return c


def kernel(H, A, B):
    global LAST_RESULTS
    from concourse.bass_utils import run_bass_kernel_spmd

    nc = _get_program()
    consts = _make_consts()

    in_maps = []
    for b in range(B_):
        cst = consts.copy()
        cst[:, 0:128] = np.asarray(B, dtype=np.float32)
        in_maps.append(
            {
                "at": np.ascontiguousarray(np.asarray(A[b], dtype=np.float32).T),
                "ht": np.ascontiguousarray(np.asarray(H[b], dtype=np.float32).T),
                "consts": cst,
            }
        )

    res = run_bass_kernel_spmd(nc, in_maps, list(range(N_CORES)))
    LAST_RESULTS = res

    out = np.empty((B_, N_, O_), dtype=np.float32)
    for b in range(B_):
        out[b] = res.results[b]["out"]
    return out


# revision 6
# speedup vs baseline: 1.2642x; 1.1407x over previous
"""GCN layer kernel for Trainium2: out[b] = D^-1/2 (A[b]+I) D^-1/2 H[b] B.

Data-parallel, one graph per NeuronCore, no collectives.

Refactoring (never materializes the normalized adjacency):
    P = H @ B;  X = d ⊙rows P;  Y = A @ X + X;  out = d ⊙rows Y
with d = 1/sqrt(1 + rowsum(A)).

Device works in transposed space: host passes AT = A[b].T, HT = H[b].T (pure
layout prep), PE computes YT = X^T @ A^T (+ X^T via identity matmul)
contracting over SBUF partitions, the epilogue scales YT columns by a
broadcast d built from a PE outer product, and the host transposes the
[O, N] result back.

deg (= rowsum A = colsum AT) comes from ones-weight matmuls overlapping the
AT DMA stream; chunk sizes taper (4,4,4,2,1,1 slabs) so the final chunk's
deg matmuls add only ~2us after the last DMA byte. All matmuls are float32r
(full fp32 element precision at 1 cycle/row, verified on HW). rsqrt runs
per-128-column on a transposed [128,1] layout so the first X tile is ready
~1us after deg completes.
"""
import sys

sys.path.insert(0, "/opt/trn_rl_repo")

import numpy as np

B_, N_, F_, O_ = 8, 2048, 128, 128
NT = N_ // 128  # 16 slabs of AT
CHUNKS = [4, 4, 4, 2, 1, 1]  # slabs per DMA chunk (tapered tail)
N_CORES = 8

_CACHE = {}
LAST_RESULTS = None


def _build_program():
    import concourse.bacc as bacc
    import concourse.tile as tile
    import concourse.mybir as mybir

    f32 = mybir.dt.float32
    f32r = mybir.dt.float32r
    AF = mybir.ActivationFunctionType

    nc = bacc.Bacc(None, target_bir_lowering=False)
    AT = nc.dram_tensor("at", [N_, N_], f32r, kind="ExternalInput")
    HT = nc.dram_tensor("ht", [F_, N_], f32r, kind="ExternalInput")
    # consts: [bw | eye | ones | ones_col]
    CST = nc.dram_tensor("consts", [128, 385], f32r, kind="ExternalInput")
    OT = nc.dram_tensor("ot", [O_, N_], f32, kind="ExternalOutput")

    at_view = AT.rearrange("(s p) i -> p s i", p=128)  # [128, NT, N_]

    chunk_start = []
    s0 = 0
    for csz in CHUNKS:
        chunk_start.append(s0)
        s0 += csz

    with tile.TileContext(nc) as tc:
        with (
            tc.tile_pool(name="const", bufs=1) as cst,
            tc.tile_pool(name="achunks", bufs=1) as ach,
            tc.tile_pool(name="small", bufs=1) as sml,
            tc.tile_pool(name="outp", bufs=2) as outp,
            tc.tile_pool(name="psbig", bufs=1, space="PSUM") as psb,
            tc.tile_pool(name="pssmall", bufs=3, space="PSUM") as pss,
        ):
            cst_sb = cst.tile([128, 385], f32r, tag="cst")
            ht_sb = cst.tile([128, N_], f32r, tag="ht")
            nc.sync.dma_start(out=cst_sb, in_=CST[:, :])
            nc.sync.dma_start(out=ht_sb, in_=HT[:, :])
            bw = cst_sb[:, 0:128]
            eye = cst_sb[:, 128:256]
            ones = cst_sb[:, 256:384]
            onesf = cst_sb[:, 384:385].bitcast(f32)
            eyef = cst_sb[:, 128:256].bitcast(f32)

            # A^T resident chunks; all DMAs issued up-front (FIFO on SP ring)
            at_slab = [None] * NT
            for ci, csz in enumerate(CHUNKS):
                st = chunk_start[ci]
                t = ach.tile([128, csz, N_], f32r, tag=f"at{ci}")
                nc.sync.dma_start(out=t, in_=at_view[:, st : st + csz, :])
                for sl in range(csz):
                    at_slab[st + sl] = t[:, sl, :]

            # P = H @ B, evacuated to SBUF unscaled (fp32)
            p_sb = sml.tile([128, NT, O_], f32, tag="p")
            for t in range(NT):
                p_ps = pss.tile([128, O_], f32, tag="sm")
                nc.tensor.matmul(
                    p_ps, ht_sb[:, t * 128 : (t + 1) * 128], bw, start=True, stop=True
                )
                nc.vector.tensor_copy(p_sb[:, t, :], p_ps)

            # deg broadcast: ones.T @ AT accumulated over slabs, overlaps DMA
            deg_ps = psb.tile([128, N_], f32, tag="big")
            for s in range(NT):
                for ib in range(4):
                    nc.tensor.matmul(
                        deg_ps[:, ib * 512 : (ib + 1) * 512],
                        ones,
                        at_slab[s][:, ib * 512 : (ib + 1) * 512],
                        start=(s == 0),
                        stop=(s == NT - 1),
                    )

            # d-chain, pipelined per 512-chunk of deg: sqrt on ACT row 0,
            # PE-transpose each 128-chunk to [128,1], reciprocal per column,
            # and scale that column's X tile immediately.
            dgsq_sb = sml.tile([1, N_], f32, tag="dgsq")
            d_sb = sml.tile([128, NT], f32, tag="d")
            xs = []
            for t in range(NT):
                x_t = sml.tile([128, O_], f32r, tag=f"x{t}")
                xs.append(x_t)
            for q in range(4):
                nc.scalar.activation(
                    out=dgsq_sb[:, q * 512 : (q + 1) * 512],
                    in_=deg_ps[0:1, q * 512 : (q + 1) * 512],
                    func=AF.Sqrt,
                    bias=1.0,
                    scale=1.0,
                )
                for t in range(q * 4, q * 4 + 4):
                    tp_ps = pss.tile([128, 1], f32, tag="sm")
                    nc.tensor.transpose(
                        tp_ps, dgsq_sb[0:1, t * 128 : (t + 1) * 128], onesf[0:1, 0:1]
                    )
                    nc.vector.tensor_copy(d_sb[:, t : t + 1], tp_ps)
                    nc.vector.reciprocal(
                        out=d_sb[:, t : t + 1], in_=d_sb[:, t : t + 1]
                    )
                    nc.vector.tensor_scalar_mul(
                        xs[t], p_sb[:, t, :], d_sb[:, t : t + 1]
                    )

            # broadcast d over partitions: transpose d_sb -> [16,128], flatten
            # to a [1, 2048] row via a tiny SWDGE DMA (16x512B descriptors),
            # then 4 outer-product matmuls ones[1,128]^T @ d_row -> [128,512]
            dT_ps = pss.tile([16, 128], f32, tag="sm")
            nc.tensor.transpose(dT_ps, d_sb, eyef)
            dT_sb = sml.tile([16, 128], f32, tag="dT")
            nc.vector.tensor_copy(dT_sb, dT_ps)
            d_row = sml.tile([1, N_], f32r, tag="drow")
            nc.gpsimd.dma_start(
                out=d_row[0:1, :].rearrange("a (t p) -> a t p", t=16),
                in_=dT_sb[:, :],
            )

            yt_ps = psb.tile([128, N_], f32, tag="big")
            dbc_sb = sml.tile([128, N_], f32, tag="dbc")

            def emit_mms(ib):
                blk = slice(ib * 512, (ib + 1) * 512)
                for t in range(NT):
                    nc.tensor.matmul(
                        yt_ps[:, blk],
                        xs[t],
                        at_slab[t][:, ib * 512 : (ib + 1) * 512],
                        start=(t == 0),
                        stop=False,
                    )
                for c in range(4):
                    cc = ib * 4 + c
                    nc.tensor.matmul(
                        yt_ps[:, cc * 128 : (cc + 1) * 128],
                        xs[cc],
                        eye,
                        start=False,
                        stop=(c == 3),
                    )

            def emit_outer():
                for q in range(4):
                    obc_ps = pss.tile([128, 512], f32, tag="sm")
                    nc.tensor.matmul(
                        obc_ps,
                        ones[0:1, 0:128],
                        d_row[0:1, q * 512 : (q + 1) * 512],
                        start=True,
                        stop=True,
                    )
                    nc.vector.tensor_copy(dbc_sb[:, q * 512 : (q + 1) * 512], obc_ps)

            def emit_tail(ib):
                blk = slice(ib * 512, (ib + 1) * 512)
                ost = outp.tile([128, 512], f32, tag="ost")
                nc.vector.tensor_mul(ost, yt_ps[:, blk], dbc_sb[:, blk])
                nc.sync.dma_start(out=OT[:, blk], in_=ost)

            for ib in range(4):
                emit_mms(ib)
                if ib == 0:
                    emit_outer()
                else:
                    emit_tail(ib - 1)
            emit_tail(3)

    nc.compile()
    return nc


def _get_program():
    if "nc" not in _CACHE:
        _CACHE["nc"] = _build_program()
    return _CACHE["nc"]


def _make_consts():
    c = np.zeros((128, 385), dtype=np.float32)
    c[:, 128:256] = np.eye(128, dtype=np.float32)
    c[:, 256:384] = 1.0
    c[:, 384] = 1.0
    return c


def kernel(H, A, B):
    global LAST_RESULTS
    from concourse.bass_utils import run_bass_kernel_spmd

    nc = _get_program()
    consts = _make_consts()

    in_maps = []
    for b in range(B_):
        cst = consts.copy()
        cst[:, 0:128] = np.asarray(B, dtype=np.float32)
        in_maps.append(
            {
                "at": np.ascontiguousarray(np.asarray(A[b], dtype=np.float32).T),
                "ht": np.ascontiguousarray(np.asarray(H[b], dtype=np.float32).T),
                "consts": cst,
            }
        )

    res = run_bass_kernel_spmd(nc, in_maps, list(range(N_CORES)))
    LAST_RESULTS = res

    out = np.empty((B_, N_, O_), dtype=np.float32)
    for b in range(B_):
        out[b] = res.results[b]["ot"].T
    return out
